# revision 5
# baseline (speedup 1.0000x reference)
"""Trainium2 Bass kernel for nn_Message (GNN message MLP).

Computes, for each of B*E rows:
    x = concat(h[64], b_in, b_out, J, -J)           # [68]
    out = relu(relu(x@W1+b1)@W2+b2)@W3 + b3         # [5]

Sharding: data-parallel over the batch dim B=8 -> one NeuronCore per batch.

Per-core device layout ("stacked pair" scheme):
  - h rows are loaded natural ([128 edges, 64 feats]) and PE-transposed in
    [128,128] squares, giving a K-major tile [128, N] whose partitions 0..63
    hold features of edge-half A and 64..127 hold edge-half B.
  - The MLP runs as quadrant-packed matmuls (tile_position) so each PSUM
    tile [128, N] carries two independent 128-edge halves.
  - The extra features (b_in, b_out, J, with -J folded into the weights) are
    host-packed into dense [3, E/2] arrays and enter as small K=3 matmuls
    accumulating into the same PSUM banks from otherwise-idle PE quadrants.
  - Biases b1/b2 are applied by the ScalarE relu evacuations (per-partition
    bias), b3 by the VectorE output evacuation.
  - The output leaves the device transposed ([5, n]) and is unshuffled on the
    host during the unshard step.
"""

import numpy as np

import concourse.bass as bass
import concourse.mybir as mybir
import concourse.tile as tile
import concourse.bass_utils as bass_utils
from concourse.masks import make_identity
from concourse.vector_clock import ScopedClock

# ---------------------------------------------------------------------------
# Workaround: this walrus build rejects >1 sync-wait on a CTRL (Drain/NoOp)
# instruction.  TileContext's exit drain accumulates one wait per DMA-sem
# lane; split the extras onto one NoOp each.
# ---------------------------------------------------------------------------


def _patched_drain_and_barrier(self, tick_clock, wait_clock):
    nc = self.nc
    drain_inst = nc.sync.drain()
    wait_clock.add_sem_waits(
        drain_inst.ins, ScopedClock({None: tick_clock.global_clock})
    )
    si = drain_inst.ins.sync_info
    if si is not None and len(si.on_wait) > 1:
        waits = list(si.on_wait)
        drain_inst.ins.sync_info = mybir.SyncInfo(
            on_wait=[waits[0]], on_update=list(si.on_update)
        )
        for w in waits[1:]:
            nop = nc.sync.nop()
            nop.ins.sync_info = mybir.SyncInfo(on_wait=[w], on_update=[])

    nc.all_engine_barrier()
    assert self.sems is not None
    popped = nc._tile_sem_poison_stack.pop()
    assert popped is self._sem_poison
    nc.clear_and_free_semaphores(list(self.sems.allocated().values()))
    nc.all_engine_barrier()


tile.TileContext._drain_and_barrier = _patched_drain_and_barrier

# ---------------------------------------------------------------------------
# Same walrus limitation, general form: at most ONE sync-wait per instruction.
# Tile's add_semaphores can emit several on one instruction (one per sem lane).
# Legalize the serialized BIR: hoist all-but-the-last wait of any instruction
# onto freshly inserted same-engine NoOps placed immediately before it.
# ---------------------------------------------------------------------------
import json as _json

_orig_to_json_bytes = bass.Bass.to_json_bytes


def _split_multi_waits(module):
    counter = [0]

    def fix_instr_list(instrs):
        out = []
        for inst in instrs:
            si = inst.get("sync_info")
            if si:
                waits = si.get("on_wait") or []
                if len(waits) > 1:
                    for w in waits[:-1]:
                        counter[0] += 1
                        out.append({
                            "engine": inst["engine"],
                            "ins": [],
                            "outs": [],
                            "name": f"LGW-{counter[0]}-{inst['name']}",
                            "opcode": "NoOp",
                            "sync_info": {"on_update": [], "on_wait": [w]},
                            **({"debug": inst["debug"]}
                               if "debug" in inst else {}),
                        })
                    si["on_wait"] = [waits[-1]]
            out.append(inst)
        return out

    def walk(o):
        if isinstance(o, dict):
            for k, v in o.items():
                if (k == "instructions" and isinstance(v, list) and v
                        and isinstance(v[0], dict) and "opcode" in v[0]):
                    o[k] = fix_instr_list(v)
                else:
                    walk(v)
        elif isinstance(o, list):
            for v in o:
                walk(v)

    walk(module)
    return module


def _patched_to_json_bytes(self, *args, **kwargs):
    raw = _orig_to_json_bytes(self, *args, **kwargs)
    module = _json.loads(raw)
    module = _split_multi_waits(module)
    return _json.dumps(module).encode()


bass.Bass.to_json_bytes = _patched_to_json_bytes

# ---------------------------------------------------------------------------
# Problem constants (hardcoded per the harness contract).
# ---------------------------------------------------------------------------
B = 8
E = 200000
HID = 64
NCORES = 8

NPAIR = E // 256          # 781 complete 256-edge pairs
EHALF = NPAIR * 128       # 99968 packed half-length
TAIL = E - NPAIR * 256    # 64 leftover edges
BIGLOAD = 4096            # edges per h DMA (16 pairs, 4 sub-groups)
SUBN = 512                # free-dim columns per compute sub-group (4 pairs)

F32 = mybir.dt.float32

_cached = {}


def _build_nc():
    nc = bass.Bass("TRN2", target_bir_lowering=False, debug=False,
                   num_devices=NCORES)

    h_d = nc.dram_tensor("h", [E, HID], F32, kind="ExternalInput")
    eA_d = nc.dram_tensor("eA", [3, EHALF], F32, kind="ExternalInput")
    eB_d = nc.dram_tensor("eB", [3, EHALF], F32, kind="ExternalInput")
    et_d = nc.dram_tensor("et", [3, TAIL], F32, kind="ExternalInput")
    W1s_d = nc.dram_tensor("W1s", [128, 64], F32, kind="ExternalInput")
    W1e_d = nc.dram_tensor("W1e", [128, 64], F32, kind="ExternalInput")
    W2s_d = nc.dram_tensor("W2s", [128, 64], F32, kind="ExternalInput")
    W3s_d = nc.dram_tensor("W3s", [128, 5], F32, kind="ExternalInput")
    b1s_d = nc.dram_tensor("b1s", [128, 1], F32, kind="ExternalInput")
    b2s_d = nc.dram_tensor("b2s", [128, 1], F32, kind="ExternalInput")
    b3s_d = nc.dram_tensor("b3s", [64, 1], F32, kind="ExternalInput")

    outA_d = nc.dram_tensor("outA", [5, EHALF], F32, kind="ExternalOutput")
    outB_d = nc.dram_tensor("outB", [5, EHALF], F32, kind="ExternalOutput")
    outT_d = nc.dram_tensor("outT", [5, TAIL], F32, kind="ExternalOutput")

    RELU = mybir.ActivationFunctionType.Relu

    with tile.TileContext(nc) as tc:
        with (
            tc.tile_pool(name="singles", bufs=1) as singles,
            tc.tile_pool(name="inp", bufs=3) as in_pool,
            tc.tile_pool(name="ext", bufs=3) as ext_pool,
            tc.tile_pool(name="acts", bufs=3) as act_pool,
            tc.tile_pool(name="outs", bufs=3) as out_pool,
            tc.tile_pool(name="ps", bufs=2, space="PSUM") as ps_pool,
        ):
            ident = singles.tile([128, 128], F32)
            make_identity(nc, ident[:])
            W1s = singles.tile([128, 64], F32)
            nc.sync.dma_start(out=W1s[:], in_=W1s_d.ap())
            W1e = singles.tile([128, 64], F32)
            nc.sync.dma_start(out=W1e[:], in_=W1e_d.ap())
            W2s = singles.tile([128, 64], F32)
            nc.sync.dma_start(out=W2s[:], in_=W2s_d.ap())
            W3s = singles.tile([128, 5], F32)
            nc.sync.dma_start(out=W3s[:], in_=W3s_d.ap())
            b1s = singles.tile([128, 1], F32)
            nc.sync.dma_start(out=b1s[:], in_=b1s_d.ap())
            b2s = singles.tile([128, 1], F32)
            nc.sync.dma_start(out=b2s[:], in_=b2s_d.ap())
            b3s = singles.tile([64, 1], F32)
            nc.sync.dma_start(out=b3s[:], in_=b3s_d.ap())

            def h_stacked_ap(e0, nedges):
                """AP over h[e0:e0+nedges] as [128, C, 64]: partition p =
                edge (mod 128), chunk c -> edges e0+128c..; matches the
                transpose input layout [p, (c f)]."""
                base = h_d.ap()
                return bass.AP(
                    tensor=base.tensor,
                    offset=base.offset + e0 * HID,
                    ap=[[HID, 128], [128 * HID, nedges // 128], [1, HID]],
                )

            def emit_block(pair0, npairs, in_tile, col0):
                """MLP over npairs stacked 256-edge pairs from in_tile."""
                n = 128 * npairs
                c0 = 128 * pair0

                ext = ext_pool.tile([67, SUBN], F32, tag="ext")
                nc.sync.dma_start(out=ext[64:67, :n],
                                  in_=eA_d.ap()[:, c0:c0 + n])
                nc.sync.dma_start(out=ext[0:3, :n],
                                  in_=eB_d.ap()[:, c0:c0 + n])

                xTp = ps_pool.tile([128, SUBN], F32, tag="xTp")
                for k in range(npairs):
                    nc.tensor.transpose(
                        xTp[:, 128 * k:128 * (k + 1)],
                        in_tile[:, col0 + 128 * k:col0 + 128 * (k + 1)],
                        ident[:],
                    )
                xT = act_pool.tile([128, SUBN], F32, tag="xT")
                nc.vector.tensor_copy(xT[:, :n], xTp[:, :n])

                ps1 = ps_pool.tile([128, SUBN], F32, tag="ps1")
                nc.tensor.matmul(ps1[0:64, :n], W1s[0:64, :], xT[0:64, :n],
                                 start=True, stop=False,
                                 tile_position=(0, 0))
                nc.tensor.matmul(ps1[0:64, :n], W1e[64:67, :],
                                 ext[64:67, :n],
                                 start=False, stop=True,
                                 tile_position=(64, 0))
                nc.tensor.matmul(ps1[64:128, :n], W1s[64:128, :],
                                 xT[64:128, :n],
                                 start=True, stop=False,
                                 tile_position=(64, 64))
                nc.tensor.matmul(ps1[64:128, :n], W1e[0:3, :], ext[0:3, :n],
                                 start=False, stop=True,
                                 tile_position=(0, 64))

                h1 = act_pool.tile([128, SUBN], F32, tag="h1")
                nc.scalar.activation(h1[:, :n], ps1[:, :n], RELU,
                                     bias=b1s[:, 0:1])

                ps2 = ps_pool.tile([128, SUBN], F32, tag="ps2")
                nc.tensor.matmul(ps2[0:64, :n], W2s[0:64, :], h1[0:64, :n],
                                 tile_position=(0, 0))
                nc.tensor.matmul(ps2[64:128, :n], W2s[64:128, :],
                                 h1[64:128, :n],
                                 tile_position=(64, 64))

                h2 = act_pool.tile([128, SUBN], F32, tag="h2")
                nc.scalar.activation(h2[:, :n], ps2[:, :n], RELU,
                                     bias=b2s[:, 0:1])

                ps3 = ps_pool.tile([37, SUBN], F32, tag="ps3")
                nc.tensor.matmul(ps3[0:5, :n], W3s[0:64, :], h2[0:64, :n],
                                 tile_position=(0, 0))
                nc.tensor.matmul(ps3[32:37, :n], W3s[64:128, :],
                                 h2[64:128, :n],
                                 tile_position=(64, 32))

                osb = out_pool.tile([37, SUBN], F32, tag="osb")
                nc.vector.tensor_scalar_add(osb[:, :n], ps3[:, :n],
                                            b3s[0:37, 0:1])

                nc.sync.dma_start(out=outA_d.ap()[:, c0:c0 + n],
                                  in_=osb[0:5, :n])
                nc.sync.dma_start(out=outB_d.ap()[:, c0:c0 + n],
                                  in_=osb[32:37, :n])

            # Main region: big h loads of 4096 edges (16 pairs each).
            nfull_big = (NPAIR * 256) // BIGLOAD          # 48
            for bl in range(nfull_big):
                e0 = bl * BIGLOAD
                in_h = in_pool.tile([128, BIGLOAD // 2], F32, tag="in_h")
                nc.sync.dma_start(out=in_h[:], in_=h_stacked_ap(e0, BIGLOAD))
                for sg in range(BIGLOAD // 1024):
                    emit_block(e0 // 256 + 4 * sg, 4, in_h, 512 * sg)

            # Remaining complete pairs (199936-196608 = 3328 edges = 13 pairs)
            e0 = nfull_big * BIGLOAD
            rem_pairs = NPAIR - e0 // 256
            if rem_pairs:
                rem_edges = rem_pairs * 256
                in_h = in_pool.tile([128, BIGLOAD // 2], F32, tag="in_h")
                nc.sync.dma_start(out=in_h[:, :rem_edges // 2],
                                  in_=h_stacked_ap(e0, rem_edges))
                p0 = e0 // 256
                done = 0
                while done < rem_pairs:
                    np_here = min(4, rem_pairs - done)
                    emit_block(p0 + done, np_here, in_h, 128 * done)
                    done += np_here

            # Tail: last 64 edges, single 64x64 transpose, no pair stacking.
            if TAIL:
                e0 = NPAIR * 256
                in_t = in_pool.tile([64, 64], F32, tag="in_t")
                nc.sync.dma_start(out=in_t[:], in_=h_d.ap()[e0:e0 + TAIL, :])
                ext_t = ext_pool.tile([67, 64], F32, tag="ext_t")
                nc.sync.dma_start(out=ext_t[64:67, :], in_=et_d.ap())

                xTp = ps_pool.tile([128, SUBN], F32, tag="xTp")
                nc.tensor.transpose(xTp[0:64, 0:64], in_t[:],
                                    ident[0:64, 0:64])
                xT = act_pool.tile([128, SUBN], F32, tag="xT")
                nc.vector.tensor_copy(xT[0:64, 0:64], xTp[0:64, 0:64])

                ps1 = ps_pool.tile([128, SUBN], F32, tag="ps1")
                nc.tensor.matmul(ps1[0:64, 0:64], W1s[0:64, :],
                                 xT[0:64, 0:64],
                                 start=True, stop=False,
                                 tile_position=(0, 0))
                nc.tensor.matmul(ps1[0:64, 0:64], W1e[64:67, :],
                                 ext_t[64:67, :],
                                 start=False, stop=True,
                                 tile_position=(64, 0))
                h1 = act_pool.tile([128, SUBN], F32, tag="h1")
                nc.scalar.activation(h1[0:64, 0:64], ps1[0:64, 0:64], RELU,
                                     bias=b1s[0:64, 0:1])
                ps2 = ps_pool.tile([128, SUBN], F32, tag="ps2")
                nc.tensor.matmul(ps2[0:64, 0:64], W2s[0:64, :],
                                 h1[0:64, 0:64], tile_position=(0, 0))
                h2 = act_pool.tile([128, SUBN], F32, tag="h2")
                nc.scalar.activation(h2[0:64, 0:64], ps2[0:64, 0:64], RELU,
                                     bias=b2s[0:64, 0:1])
                ps3 = ps_pool.tile([37, SUBN], F32, tag="ps3")
                nc.tensor.matmul(ps3[0:5, 0:64], W3s[0:64, :],
                                 h2[0:64, 0:64], tile_position=(0, 0))
                osb = out_pool.tile([37, SUBN], F32, tag="osb")
                nc.vector.tensor_scalar_add(osb[0:5, 0:64], ps3[0:5, 0:64],
                                            b3s[0:5, 0:1])
                nc.sync.dma_start(out=outT_d.ap(), in_=osb[0:5, 0:64])

    return nc


def _get_nc():
    if "nc" not in _cached:
        _cached["nc"] = _build_nc()
    return _cached["nc"]


def _pack_inputs(h, J, b_in, b_out, W1, b1, W2, b2, W3, b3):
    """Host-side shard + repack. Returns list of per-core input dicts."""
    f32 = np.float32
    # -J folded: effective J row = W1[66] - W1[67]
    W1h = np.ascontiguousarray(W1[:64]).astype(f32)            # [64, 64]
    W1ex = np.stack([W1[64], W1[65], W1[66] - W1[67]]).astype(f32)  # [3, 64]
    W1s = np.concatenate([W1h, W1h], 0)                        # [128, 64]
    W1e = np.zeros((128, 64), f32)
    W1e[0:3] = W1ex
    W1e[64:67] = W1ex
    W2s = np.concatenate([W2, W2], 0).astype(f32)              # [128, 64]
    W3s = np.concatenate([W3, W3], 0).astype(f32)              # [128, 5]
    b1s = np.concatenate([b1, b1]).astype(f32).reshape(128, 1)
    b2s = np.concatenate([b2, b2]).astype(f32).reshape(128, 1)
    b3s = np.zeros((64, 1), f32)
    b3s[0:5, 0] = b3
    b3s[32:37, 0] = b3

    in_maps = []
    for b in range(B):
        eA = np.empty((3, EHALF), f32)
        eB = np.empty((3, EHALF), f32)
        et = np.empty((3, TAIL), f32)
        for j, arr in enumerate((b_in, b_out, J)):
            v = np.asarray(arr[b, :, 0], f32)
            m = v[:NPAIR * 256].reshape(NPAIR, 2, 128)
            eA[j] = m[:, 0].ravel()
            eB[j] = m[:, 1].ravel()
            et[j] = v[NPAIR * 256:]
        in_maps.append({
            "h": np.ascontiguousarray(h[b], f32),
            "eA": eA, "eB": eB, "et": et,
            "W1s": W1s, "W1e": W1e, "W2s": W2s, "W3s": W3s,
            "b1s": b1s, "b2s": b2s, "b3s": b3s,
        })
    return in_maps


def _unpack_outputs(results):
    out = np.empty((B, E, 5), np.float32)
    for b in range(B):
        r = results[b]
        oA = r["outA"].reshape(5, NPAIR, 128)
        oB = r["outB"].reshape(5, NPAIR, 128)
        main = out[b, :NPAIR * 256].reshape(NPAIR, 2, 128, 5)
        main[:, 0] = oA.transpose(1, 2, 0)
        main[:, 1] = oB.transpose(1, 2, 0)
        out[b, NPAIR * 256:] = r["outT"].T
    return out


def run(inputs, trace=False, trace_kwargs=None):
    """Run on all 8 cores; returns (full_output, BassKernelResults)."""
    nc = _get_nc()
    in_maps = _pack_inputs(**inputs)
    kw = dict(trace_kwargs or {})
    res = bass_utils.run_bass_kernel_spmd(
        nc, in_maps, core_ids=list(range(NCORES)), trace=trace, **kw
    )
    return _unpack_outputs(res.results), res


def kernel(**inputs) -> np.ndarray:
    out, _ = run(inputs, trace=False)
    return out
